# revision 50
# baseline (speedup 1.0000x reference)
"""CrossAttention kernel for 8x Trainium2 NeuronCores (Bass/Tile).

Reference computation (per batch b):
    q = rope(x @ Wq + bq)  [L, D] -> heads [H, L, HD]
    k = enc @ Wk + bk      [LE, D] -> [H, LE, HD]
    v = enc @ Wv + bv
    out = softmax(q k^T / sqrt(HD)) v  -> concat heads -> @ Wo + bo

Sharding: DP=4 over batch x TP=2 over head-groups. Core c handles batch
(c % 4) and heads [ (c//4)*8 , (c//4)*8+8 ). Each core produces TWO
partial [L, D] outputs (outA: out-proj over the heads normalized early;
outB: the 1-2 head tail gated on the last softmax normalize); the host
sums the four partials per batch and adds bo + bv@Wo (the V bias is a
constant row vector through softmax, so it never touches the device).

Device-side layout choices (all matmuls bf16 inputs, fp32 PSUM accum):
  - host passes x^T and enc^T so the contraction dim is already on
    partitions; no on-device transposes needed anywhere.
  - scores are computed transposed (S^T[m, l]) so that P^T = exp(S^T) is
    directly the moving operand of the ctx^T matmul with V as stationary.
  - softmax skips max-subtraction: scores are ~N(0,1) bounded by ~6 for
    this problem's input distribution, exp is safe in fp32/bf16.
  - 1/sqrt(HD) and the rope pair-sign are baked into host-built cos/sin
    tables; rope pair-swap is a DVE stream_shuffle (mask swaps adjacent
    partitions within each 32-lane quadrant).

Schedule notes (perfetto-profiled):
  - ~4.5us of dummy warm-up matmuls flip the HAM clock gate to 8/8 (2.4
    GHz) during the ~8us engine preamble + first input DMAs, so real
    matmuls run warm from the start. All input-wire stalls are kept
    under the 3.4us HAM MID window so the clock never re-throttles.
  - the DMA stream is ordered by the smallest prefix that unlocks
    sustained PE work: wk half 0 + encT window 0 (interleaved at
    kc-third granularity) unlock K-proj window 0 at ~12us; Q-proj
    follows, paced chunk-by-chunk by the xT/wq stream.
  - Wo streams into the SBUF slot of the (dead after Q-proj) Wq tile.
  - exp runs on ACT over [128, 1024] tiles (2 PSUM banks) to amortize the
    ~352-cycle ACT instruction overhead; score matmuls for group g+2 are
    emitted before ctx matmuls of group g so the PE never waits on ACT.
  - the softmax denominator is bf16-folded per group (gpsimd early, DVE
    late), tree-folded to one [128, L] tensor, then partition-reduced
    AND partition-broadcast in ONE all-ones matmul per head, deferred
    into the next head's stream (after ctx_group(1)) where the whole
    fold chain has ~a head of slack. ctx PSUM and the denominator share
    double-buffered banks via per-element has_written, so the normalize
    only gates ctx matmuls two heads later.
  - V-proj drains on ACT (no bias on device); the second half of the V
    projection is emitted in quarters between attention heads 0-3 so
    their ACT exp time hides under V matmuls.
  - out-proj runs as a wide early partial (heads 0-5/0-6, emitted at
    late hooks of heads 6-7 and right after the last ctx matmul) plus a
    12-matmul tail; drains and output DMAs alternate ACT/DVE and the
    Sync/ACT DGE rings so no single queue serializes the endgame.
"""

import os

import numpy as np
import ml_dtypes

B, L, D = 4, 256, 2048
LE, DE = 2048, 1024
H = 16
HD = D // H  # 128
ROPE_BASE = 10000.0

P = 128
NCORES = 8
HN = H // 2          # heads per core (TP=2)
DC = HN * HD         # 1024 local head dims per core
KCQ = D // P         # 16 k-chunks for Q projection
KCE = DE // P        # 8 k-chunks for K/V projections
MC = LE // P         # 16 key chunks
MW = LE // 512       # 4 key windows for K^T projection
NW = D // 512        # 4 output column windows
LC = L // P          # 2 query-row chunks
NWARM = 12           # PE warm-up matmuls (N=512) at kernel start

BF16 = ml_dtypes.bfloat16

_CACHE = {}
LAST_RESULTS = None  # BassKernelResults of the most recent run (for test.py)


def _build_nc():
    import concourse.bass as bass  # noqa: F401
    import concourse.mybir as mybir
    import concourse.tile as tile
    from concourse import bacc

    f32 = mybir.dt.float32
    bf16 = mybir.dt.bfloat16
    AF = mybir.ActivationFunctionType
    OP = mybir.AluOpType

    nc = bacc.Bacc("TRN2", target_bir_lowering=False, debug=False)

    # xT and encT arrive host-packed in per-partition SBUF order so
    # every load is a contiguous full-rate transfer (their natural
    # layouts have 512B/1KB rows that stream at ~half wire rate)
    xT = nc.dram_tensor("xT", [P, KCQ * L], bf16, kind="ExternalInput").ap()
    encT = nc.dram_tensor("encT", [P, KCE * LE], bf16, kind="ExternalInput").ap()
    wq = nc.dram_tensor("wq", [D, DC], bf16, kind="ExternalInput").ap()
    # wk host-packed in per-partition SBUF order, column-half-major, so
    # each 1MB half is one contiguous full-rate DMA (K-proj window 0 +
    # the first wk half form the smallest prefix that unlocks the PE)
    wk = nc.dram_tensor("wk", [P, 2 * KCE * 512], bf16, kind="ExternalInput").ap()
    wv = nc.dram_tensor("wv", [DE, DC], bf16, kind="ExternalInput").ap()
    wo = nc.dram_tensor("wo", [DC, D], bf16, kind="ExternalInput").ap()
    # packed bf16 constants: cos | sin  (one DMA)
    CW = L + L
    cstb = nc.dram_tensor("cstb", [P, CW], bf16, kind="ExternalInput").ap()
    # packed f32 constants: bq | bk
    cstf = nc.dram_tensor("cstf", [P, 2 * HN], f32, kind="ExternalInput").ap()
    # two per-core partial outputs; the host sums them (plus biases)
    outA = nc.dram_tensor("outA", [L, D], bf16, kind="ExternalOutput").ap()
    outB = nc.dram_tensor("outB", [L, D], bf16, kind="ExternalOutput").ap()

    swap_mask = [i ^ 1 for i in range(32)]

    with tile.TileContext(nc) as tc:
        from contextlib import ExitStack

        with ExitStack() as ctx:
            const = ctx.enter_context(tc.tile_pool(name="const", bufs=1))
            keep = ctx.enter_context(tc.tile_pool(name="keep", bufs=1))
            work = ctx.enter_context(tc.tile_pool(name="work", bufs=2))
            att = ctx.enter_context(tc.tile_pool(name="att", bufs=2))
            ptpool = ctx.enter_context(tc.tile_pool(name="ptp", bufs=3))
            ph1 = ctx.enter_context(tc.tile_pool(name="phase1", bufs=1))
            ps_pp = ctx.enter_context(tc.tile_pool(name="ps_pp", bufs=2, space="PSUM"))
            ps_s = ctx.enter_context(tc.tile_pool(name="ps_s", bufs=2, space="PSUM"))
            # ctx accumulator (cols 0:L) and softmax denominator (cols
            # L:2L) share one double-buffered bank pair via per-element
            # has_written, so the normalize of head h only has to finish
            # before head h+2's first ctx matmul (~10us of slack).
            ps_cm = ctx.enter_context(tc.tile_pool(name="ps_cm", bufs=2, space="PSUM"))

            # --- PE warm-up: ~3.4us of dummy matmuls on a memset tile. The
            # ~8us engine preamble + first DMA latency already delay real
            # matmuls; the warm-up only needs one busy 4096-cycle window to
            # flip the HAM clock gate to 8/8. The all-ones tile doubles as
            # the partition-reduce+broadcast stationary operand for the
            # softmax denominator.
            onesm_sb = const.tile([P, 512], bf16, tag="onesm")
            nc.vector.memset(onesm_sb, 1.0)
            wps = ps_cm.tile([P, 512], f32, tag="cm")
            for i in range(NWARM):
                nc.tensor.matmul(
                    wps, lhsT=onesm_sb[:, 0:P], rhs=onesm_sb, start=True, stop=True
                )

            # --- constants: packed bf16 (cos|sin|bvbc) + tiny f32 (bq|bk)
            cstb_sb = const.tile([P, CW], bf16, tag="cstb")
            cos_sb = cstb_sb[:, 0:L]
            sin_sb = cstb_sb[:, L:2 * L]
            cstf_sb = const.tile([P, 2 * HN], f32, tag="cstf")
            bq_sb = cstf_sb[:, 0:HN]
            bk_sb = cstf_sb[:, HN:2 * HN]

            # --- persistent activation tensors ---
            kT_sb = keep.tile([P, HN, LE], bf16, tag="kT")      # K^T per head
            v_sb = keep.tile([P, MC, DC], bf16, tag="v")        # V  [m, d]
            qrot_sb = keep.tile([P, HN, L], bf16, tag="qrot")   # rope(Q)^T
            ctxn_sb = keep.tile([P, HN, L], bf16, tag="ctxn")   # normalized ctx^T

            # --- phase-1 input tiles. DMA order = PE consumption order:
            # cst, xT/wq (progressive row-chunks), then wk/encT/wv chunked
            # by COLUMN windows so K/V-proj tiles unlock as soon as their
            # window lands rather than after the whole tensor.
            # window-major so each window DMA is contiguous on BOTH
            # sides (the dst side otherwise caps the rate at ~196GB/s)
            encT_sb = ph1.tile([P, MW, KCE, 512], bf16, tag="encT")
            wk_sb = ph1.tile([P, 2, KCE, 512], bf16, tag="wk")
            wv_sb = ph1.tile([P, KCE, DC], bf16, tag="wv")
            xT_sb = ph1.tile([P, KCQ, L], bf16, tag="xT")
            wq_sb = ph1.tile([P, KCQ, DC], bf16, tag="wq")

            # DMA order = consumption order on the sync HWDGE ring (the
            # ACT ring measured ~2x slower for the big input stream, so it
            # only carries tiny tail DMAs). The smallest prefix that
            # unlocks PE work goes first, split at kc-half granularity so
            # K-proj window 0 can start on its first 4 k-chunks ~4us
            # before the full window lands; the residual mid-head stall is
            # under the 3.4us HAM window so the clock stays warm.
            def load_rows(dst, src, k0, n):
                nc.sync.dma_start(
                    dst[:, k0:k0 + n, :],
                    src[k0 * P:(k0 + n) * P, :].rearrange(
                        "(kc p) f -> p kc f", p=P),
                )

            def load_encw(w, c0=0, c1=KCE):
                # window w of host-packed encT: contiguous per partition
                nc.sync.dma_start(
                    encT_sb[:, w, c0:c1, :],
                    encT[:, (w * KCE + c0) * 512:(w * KCE + c1) * 512].rearrange(
                        "p (kc f) -> p kc f", kc=c1 - c0),
                )

            def load_wk(half, c0=0, c1=KCE):
                nc.sync.dma_start(
                    wk_sb[:, half, c0:c1, :],
                    wk[:, (half * KCE + c0) * 512:(half * KCE + c1) * 512].rearrange(
                        "p (kc f) -> p kc f", kc=c1 - c0),
                )

            # cstf rides the otherwise-idle ACT ring: it lands early (the
            # ACT table load + first K drain chain behind it) without
            # spending an issue slot on the Sync ring's critical prefix
            nc.scalar.dma_start(cstf_sb, cstf)
            load_wk(0, 0, 3)
            load_encw(0, 0, 3)
            load_wk(0, 3, 6)
            load_encw(0, 3, 6)
            load_wk(0, 6, KCE)
            load_encw(0, 6, KCE)
            load_wk(1)
            nc.sync.dma_start(
                xT_sb, xT.rearrange("p (kc f) -> p kc f", kc=KCQ)
            )
            load_rows(wq_sb, wq, 0, 4)
            load_rows(wq_sb, wq, 4, 4)
            load_rows(wq_sb, wq, 8, 8)
            nc.sync.dma_start(cstb_sb, cstb)          # cos/sin: needed at rope
            load_encw(1)
            load_encw(2)
            load_encw(3)
            load_rows(wv_sb, wv, 0, KCE)              # full wv, contiguous

            # --- Q projection, k-chunk-outer so the PE consumes input
            # chunks in DMA arrival order; K-proj window-0 tiles are
            # interleaved into the tail so the PE always has work while Q
            # chunks stream. The 4 concurrent head-pair PSUM accumulators
            # live in the two (idle until attention) ps_s slots: one
            # 512-col bank per head pair, two heads per bank via
            # per-element has_written.
            qpsA = ps_s.tile([P, 1024], f32, tag="sps")
            qpsB = ps_s.tile([P, 1024], f32, tag="sps")
            qps_of = {0: qpsA[:, 0:512], 1: qpsA[:, 512:1024],
                      2: qpsB[:, 0:512], 3: qpsB[:, 512:1024]}

            def q_block(kcs):
                for kc in kcs:
                    for hp in range(HN // 2):
                        for hh in range(2):
                            nc.tensor.matmul(
                                qps_of[hp][:, hh * L:(hh + 1) * L],
                                lhsT=wq_sb[:, kc, (2 * hp + hh) * P:(2 * hp + hh + 1) * P],
                                rhs=xT_sb[:, kc, :],
                                start=(kc == 0 and hh == 0),
                                stop=(kc == KCQ - 1 and hh == 1),
                            )

            def k_tiles(w, hs):
                for h in hs:
                    kps = ps_pp.tile([P, 512], f32, tag="pp")
                    for kc in range(KCE):
                        nc.tensor.matmul(
                            kps,
                            lhsT=wk_sb[:, h // 4, kc, (h % 4) * P:(h % 4 + 1) * P],
                            rhs=encT_sb[:, w, kc, :],
                            start=(kc == 0),
                            stop=(kc == KCE - 1),
                        )
                    nc.scalar.activation(
                        kT_sb[:, h, w * 512:(w + 1) * 512],
                        kps,
                        AF.Identity,
                        bias=bk_sb[:, h:h + 1],
                    )

            # K window 0 first (its 2MB input prefix is at the head of the
            # DMA stream, so real work starts ~13us in); Q follows, paced
            # chunk-by-chunk by the xT/wq stream while K window 1's encT
            # window arrives behind it.
            k_tiles(0, range(0, 4))
            k_tiles(0, range(4, HN))
            q_block(range(0, 16))

            # rope drain of Q
            for hp in range(HN // 2):
                for hh in range(2):
                    h = 2 * hp + hh
                    qf = work.tile([P, L], f32, tag="qf")
                    nc.scalar.activation(
                        qf, qps_of[hp][:, hh * L:(hh + 1) * L], AF.Identity,
                        bias=bq_sb[:, h:h + 1],
                    )
                    qs = work.tile([P, L], f32, tag="qs")
                    nc.vector.stream_shuffle(qs, qf, swap_mask)
                    t1 = work.tile([P, L], f32, tag="t1")
                    nc.vector.tensor_tensor(t1, qf, cos_sb, op=OP.mult)
                    t2 = work.tile([P, L], f32, tag="t2")
                    nc.vector.tensor_tensor(t2, qs, sin_sb, op=OP.mult)
                    nc.vector.tensor_tensor(qrot_sb[:, h, :], t1, t2, op=OP.add)

            # Wo streams into the dead Wq slot (the DMA waits for Q-proj's
            # last read of that slot)
            wo_sb = ph1.tile([P, HN, D], bf16, tag="wq")
            for c in range(4):
                h0 = 2 * c
                nc.sync.dma_start(
                    wo_sb[:, h0:h0 + 2, :],
                    wo[h0 * P:(h0 + 2) * P, :].rearrange(
                        "(h p) n -> p h n", p=P),
                )

            # --- K^T projection windows 1-3 (window 0 interleaved above) ---
            for w in range(1, MW):
                k_tiles(w, range(HN))

            # --- V projection (nh=0 up front; nh=1 in quarters between
            # attention heads 0-3 so their exp time hides under V matmuls).
            # bv is NOT added on device: softmax weights sum to 1, so the
            # bias contributes the constant row vector bv@Wo to the output,
            # which the host adds for free. The PSUM drain runs on ACT
            # (identity) where there is slack, keeping DVE off the V
            # critical path.
            def v_proj_chunk(nh, mcs):
                for mc in mcs:
                    vps = ps_pp.tile([P, 512], f32, tag="pp")
                    for kc in range(KCE):
                        nc.tensor.matmul(
                            vps,
                            lhsT=encT_sb[:, mc // 4, kc,
                                         (mc % 4) * P:(mc % 4 + 1) * P],
                            rhs=wv_sb[:, kc, nh * 512:(nh + 1) * 512],
                            start=(kc == 0),
                            stop=(kc == KCE - 1),
                        )
                    nc.scalar.activation(
                        v_sb[:, mc, nh * 512:(nh + 1) * 512], vps, AF.Identity
                    )

            v_proj_chunk(0, range(MC))

            # --- attention, software-pipelined across heads ---
            # Per head: score matmuls in 4-chunk groups into 2-bank PSUM
            # tiles, one wide exp per group on ACT, ctx matmuls delayed two
            # groups behind so the PE never waits on ACT. The softmax
            # denominator is bf16-folded (alternating DVE/GPSIMD) to
            # [128, L] per group, the 4 group sums tree-folded to one
            # [128, L] tensor, then a SINGLE all-ones stationary matmul per
            # head both partition-reduces AND broadcasts the sum to all 128
            # partitions of a PSUM tile. That matmul is deferred into the
            # next head's stream, so the whole fold chain has a full head
            # (~9us) of slack and the PE never waits on it. The normalize
            # of head h (reciprocal + ctx scale, both reading PSUM) is
            # emitted at the start of head h+1 so it frees the
            # ctx/denominator banks well before they are reused.
            NQ = 4                       # key-chunks per exp group
            NG = MC // NQ                # exp groups per head
            state = [dict() for _ in range(HN)]

            def bigsum_mm(h):
                st = state[h]
                nc.tensor.matmul(
                    st["cm"][:, L:2 * L], lhsT=onesm_sb[:, 0:P], rhs=st["sh"],
                    start=True, stop=True,
                )

            def norm(h):
                st = state[h]
                recip = att.tile([P, L], f32, tag="recip")
                nc.vector.reciprocal_approx_fast(recip, st["cm"][:, L:2 * L])
                nc.vector.tensor_tensor(
                    ctxn_sb[:, h, :], st["cm"][:, 0:L], recip, op=OP.mult
                )

            def emit_attention(h, mid_hook=None, late_hook=None):
                st = state[h]
                cm = ps_cm.tile([P, 2 * L], f32, tag="cm")
                st.update(cm=cm)
                ctxps = cm[:, 0:L]
                pts = [None] * NG
                t2s = [None] * NG

                def ctx_group(g):
                    pt = pts[g]
                    for q in range(NQ):
                        mc = NQ * g + q
                        nc.tensor.matmul(
                            ctxps,
                            lhsT=v_sb[:, mc, h * P:(h + 1) * P],
                            rhs=pt[:, q * L:(q + 1) * L],
                            start=(mc == 0),
                            stop=(mc == MC - 1),
                        )
                    # denominator: bf16-fold the 4 chunks to [128, L]
                    # (early groups on the slower GPSIMD which has slack;
                    # late groups on DVE so the tail fold is fast), then
                    # tree-fold the group sums to one [128, L] tensor; the
                    # single all-ones bigsum matmul is deferred into the
                    # next head's stream.
                    eng = nc.gpsimd if g < 2 else nc.vector
                    tf = att.tile([P, 2 * L], bf16, tag="tf")
                    eng.tensor_tensor(
                        tf, pt[:, 0:2 * L], pt[:, 2 * L:4 * L], op=OP.add
                    )
                    t2 = att.tile([P, L], bf16, tag="t2", bufs=4)
                    eng.tensor_tensor(t2, tf[:, 0:L], tf[:, L:2 * L], op=OP.add)
                    t2s[g] = t2
                    if g == 1:
                        s01 = att.tile([P, L], bf16, tag="s01")
                        nc.gpsimd.tensor_tensor(s01, t2s[0], t2s[1], op=OP.add)
                        st["s01"] = s01
                    elif g == NG - 1:
                        s23 = att.tile([P, L], bf16, tag="s23")
                        nc.vector.tensor_tensor(s23, t2s[2], t2s[3], op=OP.add)
                        sh = att.tile([P, L], bf16, tag="sh")
                        nc.vector.tensor_tensor(sh, st["s01"], s23, op=OP.add)
                        st["sh"] = sh

                def scores_group(g):
                    # 4 key-chunks share one 2-bank PSUM tile; the first
                    # mm per bank sets start=True (clears that bank)
                    sps = ps_s.tile([P, NQ * L], f32, tag="sps")
                    for q in range(NQ):
                        mc = NQ * g + q
                        nc.tensor.matmul(
                            sps[:, q * L:(q + 1) * L],
                            lhsT=kT_sb[:, h, mc * P:(mc + 1) * P],
                            rhs=qrot_sb[:, h, :],
                            start=(q % 2 == 0),
                            stop=(q % 2 == 1),
                        )
                    pt = ptpool.tile([P, NQ * L], bf16, tag="pt")
                    nc.scalar.activation(pt, sps, AF.Exp)
                    pts[g] = pt

                # mid_hook (V-quarters) sits right after the first score
                # group so its ACT drains queue behind only one exp; ctx
                # stays two groups behind scores. The late_hook (prev
                # head's bigsum+normalize, out-proj partials) sits after
                # ctx_group(1) so the prev head's fold chain has most of a
                # head of slack.
                scores_group(0)
                if mid_hook is not None:
                    mid_hook()
                scores_group(1)
                scores_group(2)
                ctx_group(0)
                scores_group(3)
                ctx_group(1)
                if late_hook is not None:
                    late_hook()
                ctx_group(2)
                ctx_group(3)

            # output projection is split into a large partial (as many
            # heads as are normalized by emission time) parked as bf16 in
            # the dead encT slot, and a thin tail over the remaining 1-2
            # heads plus one DVE add each. Chunks 0-3 partial over heads
            # 0-5 (emitted at head 6's mid, right after norm(5)); chunks
            # 4-7 over heads 0-6 (chunks 4-5 at head 7's mid, 6-7 right
            # after head 7's last ctx matmul, overlapping the head-7 fold
            # chain). The tail is then only 12 matmuls gated on norm(7).
            osbA = None
            A_HEADS = [6, 6, 6, 6, 7, 7, 7, 7]

            def out_chunk_a(chunks):
                for c in chunks:
                    lc, nw = divmod(c, NW)
                    ops = ps_pp.tile([P, 512], f32, tag="pp")
                    for h in range(A_HEADS[c]):
                        nc.tensor.matmul(
                            ops,
                            lhsT=ctxn_sb[:, h, lc * P:(lc + 1) * P],
                            rhs=wo_sb[:, h, nw * 512:(nw + 1) * 512],
                            start=(h == 0),
                            stop=(h == A_HEADS[c] - 1),
                        )
                    # drains alternate ACT/DVE so neither queue backs up
                    # (the last chunk lands on DVE, the faster op); the
                    # host does the final sum of the two partials
                    if c % 2 == 0:
                        nc.scalar.activation(osbA[:, c, :], ops, AF.Identity)
                    else:
                        nc.vector.tensor_copy(osbA[:, c, :], ops)
                    # chunks 0-3 go out in one coarse DMA (wire is idle at
                    # head 6); 4-5 as a pair; 6 and 7 individually so no
                    # large wire transfer sits in front of the outB tail
                    if c == 3:
                        nc.sync.dma_start(
                            outA[lc * P:(lc + 1) * P, :],
                            osbA[:, 0:NW, :],
                        )
                    elif c == 5:
                        nc.scalar.dma_start(
                            outA[P:2 * P, 0:1024], osbA[:, 4:6, :],
                        )
                    elif c >= 6:
                        # sync ring: keeps the ACT queue drains-only at the
                        # tail so B drains are never stuck behind an issue
                        nc.sync.dma_start(
                            outA[P:2 * P, (c - 4) * 512:(c - 3) * 512],
                            osbA[:, c, :],
                        )

            for h in range(HN):
                def mid(hh=h):
                    if 1 <= hh <= 4:
                        v_proj_chunk(1, range((hh - 1) * 4, hh * 4))

                def late(hh=h):
                    if hh >= 1:
                        bigsum_mm(hh - 1)
                        norm(hh - 1)
                    if hh == 6:
                        out_chunk_a(range(0, 4))
                    elif hh == 7:
                        out_chunk_a(range(4, 6))
                if h == 5:
                    # bf16 partials of the out-proj (slices 0-7: A half,
                    # 8-15: B tail); reuses the (dead after V-proj) encT
                    # SBUF slot
                    osbA = ph1.tile([P, 2 * LC * NW, 512], bf16, tag="encT")
                emit_attention(h, mid_hook=mid, late_hook=late)
            # chunk 6 (heads 0-6) runs while head 7's fold chain cooks on
            # DVE; the final reciprocal+normalize is then covered by chunk
            # 7's matmuls
            out_chunk_a(range(6, 7))
            bigsum_mm(HN - 1)
            norm(HN - 1)
            out_chunk_a(range(7, 8))

            # --- output projection tail: remaining 1-2 heads per chunk,
            # drained straight to the second partial output (host adds the
            # two partials). Chunks rotate over all 8 PSUM banks (the
            # score banks are free by now) so no chunk waits on a previous
            # chunk's drain; drains alternate DVE/ACT so the last chunk
            # never queues behind 3+ drains on one engine, and each
            # 128-row half goes out in one coarse DMA.
            osbB = osbA[:, LC * NW:2 * LC * NW, :]

            # chunks rotate over the 4 pp/cm banks; chunk c+4 only needs
            # chunk c's drain, which finishes ~1.3us earlier (the score
            # banks would be free too, but allocating them here would
            # chain on head 7's exp reads and stall the tail)
            def out_psum(c):
                if c % 4 < 2:
                    return ps_pp.tile([P, 512], f32, tag="pp", name="ops")
                return ps_cm.tile([P, 512], f32, tag="cm", name="ops")

            for lc in range(LC):
                for nw in range(NW):
                    c = lc * NW + nw
                    ops = out_psum(c)
                    for h in range(A_HEADS[c], HN):
                        nc.tensor.matmul(
                            ops,
                            lhsT=ctxn_sb[:, h, lc * P:(lc + 1) * P],
                            rhs=wo_sb[:, h, nw * 512:(nw + 1) * 512],
                            start=(h == A_HEADS[c]),
                            stop=(h == HN - 1),
                        )
                    # drains alternate ACT/DVE; all outB DMAs are emitted
                    # AFTER the drain loop (below) so neither engine queue
                    # interleaves issue slots between drains
                    if c % 2 == 0:
                        nc.scalar.activation(osbB[:, c, :], ops, AF.Identity)
                    else:
                        nc.vector.tensor_copy(osbB[:, c, :], ops)

            # outB goes out in three 256KB pairs alternating Sync/ACT
            # rings plus two 128KB singles, so the final wire transfer
            # (gated on chunk 7's drain) is as small as possible
            for pc in range(3):
                eng = nc.sync if pc % 2 == 0 else nc.scalar
                lc, nwp = divmod(pc, 2)
                eng.dma_start(
                    outB[lc * P:(lc + 1) * P, nwp * 1024:(nwp + 1) * 1024],
                    osbB[:, 2 * pc:2 * pc + 2, :],
                )
            nc.scalar.dma_start(outB[P:2 * P, 1024:1536], osbB[:, 6, :])
            nc.sync.dma_start(outB[P:2 * P, 1536:2048], osbB[:, 7, :])

    nc.compile()
    return nc


def _rope_tables():
    half = HD // 2
    inv_freq = 1.0 / (ROPE_BASE ** (np.arange(0, HD, 2, dtype=np.float64) / HD))
    pos = np.arange(L, dtype=np.float64)
    ang = pos[None, :] * inv_freq[:, None]  # [half, L]
    sc = 1.0 / np.sqrt(np.float64(HD))
    cos_t = np.empty((P, L), dtype=np.float32)
    sin_t = np.empty((P, L), dtype=np.float32)
    c = (np.cos(ang) * sc).astype(np.float32)
    s = (np.sin(ang) * sc).astype(np.float32)
    cos_t[0::2, :] = c
    cos_t[1::2, :] = c
    sin_t[0::2, :] = -s
    sin_t[1::2, :] = s
    return cos_t, sin_t


def prepare_in_maps(x, enc, Wq, bq, Wk, bk, Wv, bv, Wo):
    cos_t, sin_t = _rope_tables()

    # per-batch activations and per-head-group weights are shared between
    # cores (DP pairs share weights, TP pairs share activations) — build
    # each distinct array once and alias it into both cores' maps.
    # pack activations into per-partition SBUF order (see _build_nc):
    # xT[p, kc*L+f] = x[b][f, kc*128+p];  encT packed window-major:
    # encT[p, w*KCE*512 + kc*512 + f] = enc[b][w*512+f, kc*128+p]
    xTs = [
        np.ascontiguousarray(
            x[b].reshape(L, KCQ, P).transpose(2, 1, 0).reshape(P, KCQ * L)
        ).astype(BF16)
        for b in range(B)
    ]
    encTs = [
        np.ascontiguousarray(
            enc[b].reshape(MW, 512, KCE, P).transpose(3, 0, 2, 1).reshape(P, KCE * LE)
        ).astype(BF16)
        for b in range(B)
    ]
    cstbv = np.concatenate([cos_t, sin_t], axis=1).astype(BF16)
    grp = []
    for g in range(2):
        sl = slice(g * DC, (g + 1) * DC)
        cstfv = np.concatenate([
            np.ascontiguousarray(bq[sl].reshape(HN, P).T),
            np.ascontiguousarray(bk[sl].reshape(HN, P).T),
        ], axis=1).astype(np.float32)
        # wk packed in per-partition SBUF order, column-half-major:
        # wk[p, half*KCE*512 + kc*512 + f] = Wk[kc*128+p, half*512+f]
        wkp = np.ascontiguousarray(
            Wk[:, sl].reshape(KCE, P, 2, 512).transpose(1, 2, 0, 3).reshape(
                P, 2 * KCE * 512)
        ).astype(BF16)
        grp.append({
            "wq": np.ascontiguousarray(Wq[:, sl]).astype(BF16),
            "wk": wkp,
            "wv": np.ascontiguousarray(Wv[:, sl]).astype(BF16),
            "wo": np.ascontiguousarray(Wo[sl, :]).astype(BF16),
            "cstb": cstbv,
            "cstf": cstfv,
        })

    in_maps = []
    for c in range(NCORES):
        b = c % B
        g = c // B
        in_maps.append({"xT": xTs[b], "encT": encTs[b], **grp[g]})
    return in_maps


def kernel(x, encoder_inputs, Wq, bq, Wk, bk, Wv, bv, Wo, bo):
    global LAST_RESULTS
    from concourse.bass_utils import run_bass_kernel_spmd

    x = np.asarray(x, dtype=np.float32)
    enc = np.asarray(encoder_inputs, dtype=np.float32)
    Wq = np.asarray(Wq, dtype=np.float32)
    Wk = np.asarray(Wk, dtype=np.float32)
    Wv = np.asarray(Wv, dtype=np.float32)
    Wo = np.asarray(Wo, dtype=np.float32)
    bq = np.asarray(bq, dtype=np.float32)
    bk = np.asarray(bk, dtype=np.float32)
    bv = np.asarray(bv, dtype=np.float32)
    bo = np.asarray(bo, dtype=np.float32)

    if "nc" not in _CACHE:
        _CACHE["nc"] = _build_nc()
    nc = _CACHE["nc"]

    in_maps = prepare_in_maps(x, enc, Wq, bq, Wk, bk, Wv, bv, Wo)

    trace = bool(int(os.environ.get("KERNEL_TRACE", "0")))
    try:
        res = run_bass_kernel_spmd(
            nc, in_maps, core_ids=list(range(NCORES)), trace=trace
        )
    except ModuleNotFoundError:
        # NTFF profiling hook unavailable (axon client without antenv hooks)
        res = run_bass_kernel_spmd(
            nc, in_maps, core_ids=list(range(NCORES)), trace=False
        )
    LAST_RESULTS = res

    # constant row vector contributed by the (device-omitted) V bias:
    # softmax weights sum to 1, so each head adds bv_h, and the output
    # projection maps that to bv @ Wo. Added here with bo.
    cvec = bv.astype(np.float64) @ Wo.astype(np.float64)
    const_row = (cvec + bo.astype(np.float64)).astype(np.float32)

    out = np.empty((B, L, D), dtype=np.float32)
    for b in range(B):
        out[b] = (
            res.results[b]["outA"].astype(np.float32)
            + res.results[b]["outB"].astype(np.float32)
            + res.results[b + B]["outA"].astype(np.float32)
            + res.results[b + B]["outB"].astype(np.float32)
            + const_row[None, :]
        )
    return out

